# revision 1
# baseline (speedup 1.0000x reference)
"""Trainium2 Bass kernel for a pre-LN transformer block (B=128,T=256,C=384,H=6,D=64).

Data-parallel over batch across 8 NeuronCores (16 batches/core), processed in
pairs so the QKV and FFN1 matmuls stream a 512-wide moving operand (two
batches' tokens side by side). All matmuls run as float32r. LN gamma/beta are
folded into the weights on the host; device LN is (x - mu) * rstd via
bn_stats/bn_aggr. Attention uses the transposed-score orientation ([S,T]):
softmax denominators come from an all-ones matmul that also broadcasts them
across partitions, causal masking zeroes exp(scores) with
gpsimd.affine_select, and normalization happens during the attn@v PSUM
evacuation.
"""

import sys

if "/opt/trn_rl_repo" not in sys.path:
    sys.path.insert(0, "/opt/trn_rl_repo")

import numpy as np

import concourse.bass as bass
import concourse.mybir as mybir
import concourse.tile as tile
from concourse import bacc

# All ACT functions used here (Exp, Ln, Relu, Identity, Copy) live in the
# 'natural_log_exp_and_others' table set. Blank the other sets (preserving
# dict order, which defines act_func_set_id) so the table-load fixpoint
# settles on a single ACT_TABLE_LOAD instead of thrashing sets per batch.
_KEEP_ACT_SET = "natural_log_exp_and_others"
_orig_get_act_tables = bacc.get_activation_tables


def _one_set_tables(arch):
    t = _orig_get_act_tables(arch)
    assert _KEEP_ACT_SET in t
    return {k: (v if k == _KEEP_ACT_SET else set()) for k, v in t.items()}


bacc.get_activation_tables = _one_set_tables

F32 = mybir.dt.float32
F32R = mybir.dt.float32r
AF = mybir.ActivationFunctionType
ALU = mybir.AluOpType

B, T, C, H, D = 128, 256, 384, 6, 64
NCORES = 8
BL = B // NCORES          # batches per core
F = 4 * C                 # 1536
P = 128
TCH = T // P              # 2 token chunks
CCH = C // P              # 3 channel chunks
FCH = F // P              # 12 ffn chunks
HD = H * D                # 384
SCALE = float(C) ** -0.5  # reference scales by full model dim
EPS = 1e-5


def build_program(bl=BL, flags=frozenset(), repeat=1,
                  tr_split=False, tr_bufs=1, ps1_bufs=6, wk_bufs=2):
    """Per-core Bass program. `flags` lists nonzero bias terms
    ('qb','kb','vb','bo','b1','b2'). `repeat` wraps the whole computation in
    a hardware loop (benchmarking only)."""
    assert bl % 2 == 0
    use_qb = "qb" in flags
    use_kb = "kb" in flags
    use_vb = "vb" in flags
    use_bo = "bo" in flags
    use_b1 = "b1" in flags
    use_b2 = "b2" in flags

    nc = bacc.Bacc("TRN2", target_bir_lowering=False, debug=False,
                   num_devices=NCORES)

    x_d = nc.dram_tensor("x", [bl, T, C], F32, kind="ExternalInput")
    wq_d = nc.dram_tensor("wq", [P, CCH, HD], F32R, kind="ExternalInput")
    wk_d = nc.dram_tensor("wk", [P, CCH, HD], F32R, kind="ExternalInput")
    wv_d = nc.dram_tensor("wv", [P, CCH, HD], F32R, kind="ExternalInput")
    qb_d = nc.dram_tensor("qb", [P, CCH], F32, kind="ExternalInput")
    kb_d = nc.dram_tensor("kb", [P, CCH], F32, kind="ExternalInput")
    vb_d = nc.dram_tensor("vb", [1, HD], F32R, kind="ExternalInput")
    wo_d = nc.dram_tensor("wo", [D, H, C], F32R, kind="ExternalInput")
    bo_d = nc.dram_tensor("bo", [1, C], F32R, kind="ExternalInput")
    w1_d = nc.dram_tensor("w1", [P, CCH, F], F32R, kind="ExternalInput")
    b1_d = nc.dram_tensor("b1c", [P, FCH], F32, kind="ExternalInput")
    w2_d = nc.dram_tensor("w2", [P, FCH, C], F32R, kind="ExternalInput")
    b2_d = nc.dram_tensor("b2", [1, C], F32R, kind="ExternalInput")
    id_d = nc.dram_tensor("ident", [P, P], F32R, kind="ExternalInput")
    on_d = nc.dram_tensor("onesm", [P, P], F32R, kind="ExternalInput")
    tl_d = nc.dram_tensor("trilm", [P, P], F32R, kind="ExternalInput")
    ng_d = nc.dram_tensor("negm", [P, TCH, T], F32R, kind="ExternalInput")
    y_d = nc.dram_tensor("y", [bl, T, C], F32, kind="ExternalOutput")

    with tile.TileContext(nc) as tc:
        import contextlib
        with (
            tc.tile_pool(name="wpool", bufs=1) as wp,
            tc.tile_pool(name="work", bufs=wk_bufs) as wk_pool,
            tc.tile_pool(name="big", bufs=1) as bigp,
            (contextlib.nullcontext(None) if tr_split else
             tc.tile_pool(name="ps_tr", bufs=tr_bufs, space="PSUM")) as pstr,
            tc.tile_pool(name="ps_one", bufs=ps1_bufs, space="PSUM") as ps1,
        ):
            # ---- load weights/constants once ----
            wq = wp.tile([P, CCH, HD], F32R)
            wkk = wp.tile([P, CCH, HD], F32R)
            wv = wp.tile([P, CCH, HD], F32R)
            wo = wp.tile([D, H, C], F32R)
            w1 = wp.tile([P, CCH, F], F32R)
            w2 = wp.tile([P, FCH, C], F32R)
            ident = wp.tile([P, P], F32R)
            ones_t = wp.tile([P, P], F32R)
            trilm = wp.tile([P, P], F32R)
            negm = wp.tile([P, TCH, T], F32R)
            epsb = wp.tile([P, 1], F32)
            nc.gpsimd.memset(epsb[:], EPS)
            nc.sync.dma_start(wq[:], wq_d[:])
            nc.sync.dma_start(wkk[:], wk_d[:])
            nc.sync.dma_start(wv[:], wv_d[:])
            nc.sync.dma_start(wo[:], wo_d[:])
            nc.sync.dma_start(w1[:], w1_d[:])
            nc.sync.dma_start(w2[:], w2_d[:])
            nc.sync.dma_start(ident[:], id_d[:])
            nc.sync.dma_start(ones_t[:], on_d[:])
            nc.sync.dma_start(trilm[:], tl_d[:])
            nc.sync.dma_start(negm[:], ng_d[:])
            qb = kb = vb = bo = b1c = b2 = None
            if use_qb:
                qb = wp.tile([P, CCH], F32)
                nc.sync.dma_start(qb[:], qb_d[:])
            if use_kb:
                kb = wp.tile([P, CCH], F32)
                nc.sync.dma_start(kb[:], kb_d[:])
            if use_vb:
                vb = wp.tile([1, HD], F32R)
                nc.sync.dma_start(vb[:], vb_d[:])
            if use_bo:
                bo = wp.tile([1, C], F32R)
                nc.sync.dma_start(bo[:], bo_d[:])
            if use_b1:
                b1c = wp.tile([P, FCH], F32)
                nc.sync.dma_start(b1c[:], b1_d[:])
            if use_b2:
                b2 = wp.tile([1, C], F32R)
                nc.sync.dma_start(b2[:], b2_d[:])

            def layer_norm_T(src, dstT, i, evac_act):
                """src: [P, TCH, C] tokens-major tile. Writes (src-mu)*rstd
                transposed into dstT[:, :, i, :] ([P, CCH, 2, T] pair tile)."""
                st6 = wk_pool.tile([P, TCH, 6], F32, tag=f"st6_{i}")
                mv = wk_pool.tile([P, TCH, 2], F32, tag=f"mv_{i}")
                rstd = wk_pool.tile([P, TCH], F32, tag=f"rstd_{i}")
                for tch in range(TCH):
                    nc.vector.bn_stats(st6[:, tch, :], src[:, tch, :])
                    nc.vector.bn_aggr(mv[:, tch, :], st6[:, tch, :])
                # rstd = exp(-0.5 * ln(var + eps))
                nc.scalar.activation(rstd[:], mv[:, :, 1], AF.Ln, bias=epsb[:])
                nc.scalar.activation(rstd[:], rstd[:], AF.Exp, scale=-0.5)
                xn = wk_pool.tile([P, TCH, C], F32R, tag=f"xn_{i}", bufs=1)
                for tch in range(TCH):
                    nc.vector.tensor_scalar(
                        xn[:, tch, :], src[:, tch, :],
                        mv[:, tch, 0:1], rstd[:, tch:tch + 1],
                        ALU.subtract, ALU.mult,
                    )
                if tr_split:
                    trA = ps1.tile([P, 2, T], F32R, tag="ps1", name="trA")
                    trB = ps1.tile([P, T], F32R, tag="ps1", name="trB")

                    def _trdst(cc):
                        return trB if cc == 2 else trA[:, cc, :]
                else:
                    tr = pstr.tile([P, CCH, T], F32R, tag="tr")

                    def _trdst(cc):
                        return tr[:, cc, :]
                for tch in range(TCH):
                    for cc in range(CCH):
                        nc.tensor.transpose(
                            _trdst(cc)[:, tch * P:(tch + 1) * P],
                            xn[:, tch, cc * P:(cc + 1) * P],
                            ident[:],
                        )
                if tr_split:
                    if evac_act:
                        nc.scalar.copy(dstT[:, 0:2, i, :], trA[:])
                        nc.scalar.copy(dstT[:, 2, i, :], trB[:])
                    else:
                        nc.vector.tensor_copy(dstT[:, 0:2, i, :], trA[:])
                        nc.vector.tensor_copy(dstT[:, 2, i, :], trB[:])
                elif evac_act:
                    nc.scalar.copy(dstT[:, :, i, :], tr[:])
                else:
                    nc.vector.tensor_copy(dstT[:, :, i, :], tr[:])

            def body():
                for pb in range(bl // 2):
                    bp = (2 * pb, 2 * pb + 1)
                    xts = []
                    xnT2 = wk_pool.tile([P, CCH, 2, T], F32R, tag="xnT2")
                    for i, b in enumerate(bp):
                        xt = wk_pool.tile([P, TCH, C], F32, tag=f"xt{i}")
                        nc.sync.dma_start(
                            xt[:], x_d[b].rearrange("(tc p) c -> p tc c", p=P))
                        xts.append(xt)
                        layer_norm_T(xt, xnT2, i, evac_act=(i == 0))

                    # ---- q,k transposed [hd, (b,t)]; v natural [s, hd] ----
                    qsb2 = wk_pool.tile([P, CCH, 2, T], F32R, tag="qsb2")
                    ksb2 = wk_pool.tile([P, CCH, 2, T], F32R, tag="ksb2", bufs=1)
                    for wmat, bias_t, use_b, dst, eng in (
                        (wq, qb, use_qb, qsb2, "act"),
                        (wkk, kb, use_kb, ksb2, "dve"),
                    ):
                        for mc in range(CCH):
                            pp = ps1.tile([P, 2, T], F32, tag="ps1")
                            for kc in range(CCH):
                                nc.tensor.matmul(
                                    pp[:, :, :],
                                    wmat[:, kc, mc * P:(mc + 1) * P],
                                    xnT2[:, kc, :, :],
                                    start=(kc == 0), stop=(kc == CCH - 1),
                                )
                            if use_b:
                                nc.scalar.activation(
                                    dst[:, mc, :, :], pp[:], AF.Identity,
                                    bias=bias_t[:, mc:mc + 1])
                            elif eng == "act":
                                nc.scalar.copy(dst[:, mc, :, :], pp[:])
                            else:
                                nc.vector.tensor_copy(dst[:, mc, :, :], pp[:])

                    vsbs = []
                    for i in range(2):
                        vsb = wk_pool.tile([P, TCH, HD], F32R, tag=f"vsb{i}")
                        vsbs.append(vsb)
                        for sc in range(TCH):
                            vp = ps1.tile([P, HD], F32, tag="ps1")
                            for kc in range(CCH):
                                nc.tensor.matmul(
                                    vp[:, :],
                                    xnT2[:, kc, i, sc * P:(sc + 1) * P],
                                    wv[:, kc, :],
                                    start=(kc == 0),
                                    stop=(kc == CCH - 1 and not use_vb),
                                )
                            if use_vb:
                                nc.tensor.matmul(
                                    vp[:, :], ones_t[0:1, :], vb[0:1, :],
                                    start=False, stop=True)
                            if sc == 0:
                                nc.scalar.copy(vsb[:, sc, :], vp[:])
                            else:
                                nc.vector.tensor_copy(vsb[:, sc, :], vp[:])

                    # ---- attention per batch ----
                    xnews = []
                    for i, b in enumerate(bp):
                        e_all = bigp.tile([P, TCH, H, T], F32R, tag="e_all")
                        rbc = bigp.tile([P, H, T], F32, tag="rbc")
                        osb = wk_pool.tile([64, H, T], F32R, tag="osb", bufs=1)
                        for h in range(H):
                            hc, ho = h // 2, 64 * (h % 2)
                            sp = ps1.tile([P, TCH, T], F32, tag="ps1")
                            for sc in range(TCH):
                                nc.tensor.matmul(
                                    sp[:, sc, :],
                                    ksb2[ho:ho + D, hc, i, sc * P:(sc + 1) * P],
                                    qsb2[ho:ho + D, hc, i, :],
                                    start=True, stop=False,
                                )
                                # causal mask fused on PE: adds
                                # -1e30*max(0, s-t), so exp flushes to 0
                                nc.tensor.matmul(
                                    sp[:, sc, :], trilm[:, :], negm[:, sc, :],
                                    start=False, stop=True,
                                )
                            nc.scalar.activation(
                                e_all[:, :, h, :], sp[:], AF.Exp, scale=SCALE)
                        for pc in range(H // 2):
                            dp = ps1.tile([P, 2, T], F32, tag="ps1")
                            ops = []
                            for j in range(2):
                                h = 2 * pc + j
                                for sc in range(TCH):
                                    nc.tensor.matmul(
                                        dp[:, j, :], ones_t[:],
                                        e_all[:, sc, h, :],
                                        start=(sc == 0), stop=(sc == TCH - 1))
                                op_h = ps1.tile([D, T], F32, tag="ps1")
                                ops.append(op_h)
                                for sc in range(TCH):
                                    nc.tensor.matmul(
                                        op_h[:, :],
                                        vsbs[i][:, sc, h * D:(h + 1) * D],
                                        e_all[:, sc, h, :],
                                        start=(sc == 0), stop=(sc == TCH - 1))
                            nc.vector.reciprocal(
                                rbc[:, 2 * pc:2 * pc + 2, :], dp[:])
                            for j in range(2):
                                h = 2 * pc + j
                                nc.vector.tensor_tensor(
                                    osb[:, h, :], ops[j][:, :], rbc[0:D, h, :],
                                    ALU.mult,
                                )

                        # ---- out proj + residual ----
                        xnew = wk_pool.tile([P, TCH, C], F32, tag=f"xnew{i}")
                        xnews.append(xnew)
                        for tcc in range(TCH):
                            ap_t = ps1.tile([P, C], F32, tag="ps1")
                            for h in range(H):
                                nc.tensor.matmul(
                                    ap_t[:, :],
                                    osb[:, h, tcc * P:(tcc + 1) * P],
                                    wo[:, h, :],
                                    start=(h == 0),
                                    stop=(h == H - 1 and not use_bo))
                            if use_bo:
                                nc.tensor.matmul(
                                    ap_t[:, :], ones_t[0:1, :], bo[0:1, :],
                                    start=False, stop=True)
                            nc.vector.tensor_tensor(
                                xnew[:, tcc, :], ap_t[:, :], xts[i][:, tcc, :],
                                ALU.add)

                    # ---- LN2 -> xn2T pair ----
                    xn2T2 = wk_pool.tile([P, CCH, 2, T], F32R, tag="xn2T2")
                    for i in range(2):
                        layer_norm_T(xnews[i], xn2T2, i, evac_act=(i == 1))

                    # ---- FFN fused over the pair, streamed per f-chunk:
                    # hT chunk -> relu -> immediately accumulated into the
                    # four (batch, t-chunk) FFN2 output psums ----
                    fps = []
                    for j in range(4):
                        fp_j = ps1.tile([P, C], F32, tag="ps1", name=f"fp{j}")
                        fps.append(fp_j)
                    for mo in range(FCH):
                        hp = ps1.tile([P, 2, T], F32, tag="ps1")
                        for kc in range(CCH):
                            nc.tensor.matmul(
                                hp[:, :, :],
                                w1[:, kc, mo * P:(mo + 1) * P],
                                xn2T2[:, kc, :, :],
                                start=(kc == 0), stop=(kc == CCH - 1))
                        hsm = wk_pool.tile([P, 2, T], F32R, tag="hsm")
                        if use_b1:
                            nc.scalar.activation(
                                hsm[:], hp[:], AF.Relu, bias=b1c[:, mo:mo + 1])
                        else:
                            nc.scalar.activation(hsm[:], hp[:], AF.Relu)
                        for i in range(2):
                            for tcc in range(TCH):
                                nc.tensor.matmul(
                                    fps[2 * i + tcc][:, :],
                                    hsm[:, i, tcc * P:(tcc + 1) * P],
                                    w2[:, mo, :],
                                    start=(mo == 0),
                                    stop=(mo == FCH - 1 and not use_b2))

                    for i, b in enumerate(bp):
                        yout = wk_pool.tile([P, TCH, C], F32, tag=f"yout{i}")
                        for tcc in range(TCH):
                            fp = fps[2 * i + tcc]
                            if use_b2:
                                nc.tensor.matmul(
                                    fp[:, :], ones_t[0:1, :], b2[0:1, :],
                                    start=False, stop=True)
                            nc.vector.tensor_tensor(
                                yout[:, tcc, :], fp[:, :], xnews[i][:, tcc, :],
                                ALU.add)
                        nc.sync.dma_start(
                            y_d[b].rearrange("(tc p) c -> p tc c", p=P),
                            yout[:])

            if repeat > 1:
                with tc.For_i(0, repeat, 1):
                    body()
            else:
                body()

    nc.compile()
    return nc


def _make_negm():
    # negm[j, sc, t] moving operand; with trilm (lhsT[j, s] = 1 iff j <= s)
    # the accumulated matmul adds -BIG * #{j: j <= s_blk and cond(j, t)},
    # nonzero exactly where global s > t.
    BIG = np.float32(1e30)
    f32 = np.float32
    m = np.zeros((P, TCH, T), dtype=f32)
    jgt = np.tril(np.ones((P, P), dtype=f32), -1)  # [j, t] = 1 iff j > t
    m[:, 0, 0:P] = -BIG * jgt          # diagonal block of s-chunk 0
    m[:, 1, 0:P] = -BIG                # s-chunk 1 vs t-chunk 0: all masked
    m[:, 1, P:2 * P] = -BIG * jgt      # diagonal block of s-chunk 1
    return m


def prep_weights(Wq, Wk, Wv, Wo, bo, W1, b1, W2, b2, g1, be1, g2, be2):
    """Fold LN gamma/beta into projection weights; rearrange to SBUF layouts."""
    f32 = np.float32

    def kchunk(w, kdim):  # [K, M] -> [P, K//P, M]
        m = w.shape[1]
        return np.ascontiguousarray(
            w.reshape(kdim // P, P, m).transpose(1, 0, 2)).astype(f32)

    Wq2 = Wq.transpose(1, 0, 2).reshape(C, HD)
    Wk2 = Wk.transpose(1, 0, 2).reshape(C, HD)
    Wv2 = Wv.transpose(1, 0, 2).reshape(C, HD)
    out = {
        "wq": kchunk(g1[:, None] * Wq2, C),
        "wk": kchunk(g1[:, None] * Wk2, C),
        "wv": kchunk(g1[:, None] * Wv2, C),
        "wo": np.ascontiguousarray(
            Wo.reshape(H, D, C).transpose(1, 0, 2)).astype(f32),
        "w1": kchunk(g2[:, None] * W1, C),
        "w2": kchunk(W2, F),
        "ident": np.eye(P, dtype=f32),
        "onesm": np.ones((P, P), dtype=f32),
        "trilm": np.tril(np.ones((P, P), dtype=f32)).T.copy(),
        "negm": _make_negm(),
    }
    qb = be1 @ Wq2
    kb = be1 @ Wk2
    vb = be1 @ Wv2
    b1e = be2 @ W1 + b1
    out["qb"] = np.ascontiguousarray(qb.reshape(CCH, P).T).astype(f32)
    out["kb"] = np.ascontiguousarray(kb.reshape(CCH, P).T).astype(f32)
    out["vb"] = vb[None, :].astype(f32)
    out["bo"] = bo[None, :].astype(f32)
    out["b1c"] = np.ascontiguousarray(b1e.reshape(FCH, P).T).astype(f32)
    out["b2"] = b2[None, :].astype(f32)
    flags = set()
    for name, vec in (("qb", qb), ("kb", kb), ("vb", vb),
                      ("bo", bo), ("b1", b1e), ("b2", b2)):
        if np.any(vec != 0):
            flags.add(name)
    return out, frozenset(flags)


_PROGRAM_CACHE = {}


def _get_program(bl, flags):
    key = (bl, flags)
    if key not in _PROGRAM_CACHE:
        _PROGRAM_CACHE[key] = build_program(
            bl, flags, tr_split=True, ps1_bufs=8)
    return _PROGRAM_CACHE[key]


def kernel(x, Wq, Wk, Wv, Wo, bo, W1, b1, W2, b2, g1, be1, g2, be2, **kw):
    from concourse.bass_utils import run_bass_kernel_spmd

    args = [np.asarray(a, dtype=np.float32) for a in
            (x, Wq, Wk, Wv, Wo, bo, W1, b1, W2, b2, g1, be1, g2, be2)]
    x = args[0]
    wmap, flags = prep_weights(*args[1:])
    nc = _get_program(BL, flags)
    xs = x.reshape(NCORES, BL, T, C)
    in_maps = []
    for c in range(NCORES):
        m = {"x": np.ascontiguousarray(xs[c])}
        m.update(wmap)
        in_maps.append(m)
    res = run_bass_kernel_spmd(nc, in_maps, list(range(NCORES)), **kw)
    global _last_results
    _last_results = res
    y = np.stack([res.results[i]["y"] for i in range(NCORES)], axis=0)
    return y.reshape(B, T, C)


_last_results = None



# revision 2
# speedup vs baseline: 1.0172x; 1.0172x over previous
"""Trainium2 Bass kernel v2 for the pre-LN transformer block
(B=128,T=256,C=384,H=6,D=64), data-parallel over batch across 8 cores.

Differences vs v1:
- All matmul operands are bf16 (PSUM stays f32, residual stream f32).
  bf16 runs at 1 cycle/row at any moving width (fp32r needs >=256) and
  halves SBUF for weights and activations.
- Causal block sparsity: with T=256 split in two 128-chunks, the
  (s-chunk 1, t-chunk 0) score block is fully masked and never computed.
  Per head the scores are 3 blocks [d0=(s0,t0) diag, full=(s0,t1),
  d1=(s1,t1) diag]; the diag blocks get the -BIG upper-triangle mask from
  one matmul whose middle moving block is zero.
- Heads processed in pairs: head 2j occupies PSUM partitions 0:64 and
  2j+1 partitions 64:128 of shared denominator / attn@v accumulators, so
  the output projection contracts K=128 (3 matmuls instead of 6).
- Software pipelining: emission order A0 A1 B0 A2 C0 B1 A3 C1 ... where
  A=load+LN1+QKV, B=attention+LN2, C=FFN+store. Engines execute in-order,
  so this fills LN pipeline bubbles with independent work.
"""

import sys

if "/opt/trn_rl_repo" not in sys.path:
    sys.path.insert(0, "/opt/trn_rl_repo")

import numpy as np

import concourse.bass as bass
import concourse.mybir as mybir
import concourse.tile as tile
from concourse import bacc

_KEEP_ACT_SET = "natural_log_exp_and_others"
_orig_get_act_tables = bacc.get_activation_tables


def _one_set_tables(arch):
    t = _orig_get_act_tables(arch)
    assert _KEEP_ACT_SET in t
    return {k: (v if k == _KEEP_ACT_SET else set()) for k, v in t.items()}


bacc.get_activation_tables = _one_set_tables

F32 = mybir.dt.float32
BF16 = mybir.dt.bfloat16
FP8 = mybir.dt.float8e4
WSCL = 32.0               # fp8 weights are pre-scaled by this; descaled at evac
PM_DR = mybir.MatmulPerfMode.DoubleRow
AF = mybir.ActivationFunctionType
ALU = mybir.AluOpType

B, T, C, H, D = 128, 256, 384, 6, 64
NCORES = 8
BL = B // NCORES
F = 4 * C
P = 128
TCH = T // P              # 2
CCH = C // P              # 3
FCH = F // P              # 12
HD = H * D
HP = H // 2               # head pairs
SCALE = float(C) ** -0.5
EPS = 1e-5
BIG = 1e30


def build_program2(bl=BL, flags=frozenset(), repeat=1):
    assert bl % 2 == 0
    npairs = bl // 2
    use_qb = "qb" in flags
    use_kb = "kb" in flags
    use_vb = "vb" in flags
    use_bo = "bo" in flags
    use_b1 = "b1" in flags
    use_b2 = "b2" in flags

    nc = bacc.Bacc("TRN2", target_bir_lowering=False, debug=False,
                   num_devices=NCORES)

    x_d = nc.dram_tensor("x", [bl, T, C], F32, kind="ExternalInput")
    wq_d = nc.dram_tensor("wq", [P, CCH, HD], FP8, kind="ExternalInput")
    wk_d = nc.dram_tensor("wk", [P, CCH, HD], FP8, kind="ExternalInput")
    wv_d = nc.dram_tensor("wv", [P, CCH, HD], FP8, kind="ExternalInput")
    wo_d = nc.dram_tensor("wo", [P, HP, C], BF16, kind="ExternalInput")
    w1_d = nc.dram_tensor("w1", [P, CCH, F], FP8, kind="ExternalInput")
    w2_d = nc.dram_tensor("w2", [P, FCH // 2, 2, C], FP8, kind="ExternalInput")
    id_d = nc.dram_tensor("ident", [P, P], BF16, kind="ExternalInput")
    tl_d = nc.dram_tensor("trilm", [P, P], BF16, kind="ExternalInput")
    ng_d = nc.dram_tensor("negm2", [P, 2, P], BF16, kind="ExternalInput")
    on_d = nc.dram_tensor("ones64", [P, P], BF16, kind="ExternalInput")
    on8_d = nc.dram_tensor("ones8", [P, P], FP8, kind="ExternalInput")
    qb_d = nc.dram_tensor("qb", [P, CCH], F32, kind="ExternalInput")
    kb_d = nc.dram_tensor("kb", [P, CCH], F32, kind="ExternalInput")
    vb_d = nc.dram_tensor("vb", [1, HD], BF16, kind="ExternalInput")
    bo_d = nc.dram_tensor("bo", [1, C], BF16, kind="ExternalInput")
    b1_d = nc.dram_tensor("b1c", [P, FCH], F32, kind="ExternalInput")
    b2_d = nc.dram_tensor("b2", [1, C], BF16, kind="ExternalInput")
    y_d = nc.dram_tensor("y", [bl, T, C], F32, kind="ExternalOutput")

    with tile.TileContext(nc) as tc:
        with (
            tc.tile_pool(name="wpool", bufs=1) as wp,
            tc.tile_pool(name="work", bufs=2) as wk_pool,
            tc.tile_pool(name="ps", bufs=8, space="PSUM") as psp,
        ):
            # ---- constants / weights (loaded once, in first-use order) ----
            ident = wp.tile([P, P], BF16)
            wq = wp.tile([P, CCH, HD], FP8)
            wkk = wp.tile([P, CCH, HD], FP8)
            wv = wp.tile([P, CCH, HD], FP8)
            trilm = wp.tile([P, P], BF16)
            negm2 = wp.tile([P, 2, P], BF16)
            ones64 = wp.tile([P, P], BF16)
            ones8 = wp.tile([P, P], FP8)
            wo = wp.tile([P, HP, C], BF16)
            w1 = wp.tile([P, CCH, F], FP8)
            w2 = wp.tile([P, FCH // 2, 2, C], FP8)
            epsb = wp.tile([P, 1], F32)
            nc.gpsimd.memset(epsb[:], EPS)
            for dst, src in ((ident, id_d), (wq, wq_d), (wkk, wk_d),
                             (wv, wv_d), (trilm, tl_d), (negm2, ng_d),
                             (ones64, on_d), (ones8, on8_d),
                             (wo, wo_d), (w1, w1_d),
                             (w2, w2_d)):
                nc.sync.dma_start(dst[:], src[:])
            qb = kb = vb = bo = b1c = b2 = None
            if use_qb:
                qb = wp.tile([P, CCH], F32)
                nc.sync.dma_start(qb[:], qb_d[:])
            if use_kb:
                kb = wp.tile([P, CCH], F32)
                nc.sync.dma_start(kb[:], kb_d[:])
            if use_vb:
                vb = wp.tile([1, HD], BF16)
                nc.sync.dma_start(vb[:], vb_d[:])
            if use_bo:
                bo = wp.tile([1, C], BF16)
                nc.sync.dma_start(bo[:], bo_d[:])
            if use_b1:
                b1c = wp.tile([P, FCH], F32)
                nc.sync.dma_start(b1c[:], b1_d[:])
            if use_b2:
                b2 = wp.tile([1, C], BF16)
                nc.sync.dma_start(b2[:], b2_d[:])

            def layer_norm_pair(srcs, xns, tag):
                """token-major LN for both batches of a pair:
                xn (bf16) = (src - mu) * rstd."""
                st6 = wk_pool.tile([P, 2, TCH, 6], F32, tag=f"st6_{tag}")
                mv = wk_pool.tile([P, 2, TCH, 2], F32, tag=f"mv_{tag}")
                rstd = wk_pool.tile([P, 2, TCH], F32, tag=f"rstd_{tag}")
                for i in range(2):
                    for tch in range(TCH):
                        nc.vector.bn_stats(st6[:, i, tch, :],
                                           srcs[i][:, tch, :])
                        nc.vector.bn_aggr(mv[:, i, tch, :],
                                          st6[:, i, tch, :])
                nc.scalar.activation(rstd[:], mv[:, :, :, 1], AF.Ln,
                                     bias=epsb[:])
                nc.scalar.activation(rstd[:], rstd[:], AF.Exp, scale=-0.5)
                for i in range(2):
                    for tch in range(TCH):
                        nc.vector.tensor_scalar(
                            xns[i][:, tch, :], srcs[i][:, tch, :],
                            mv[:, i, tch, 0:1], rstd[:, i, tch:tch + 1],
                            ALU.subtract, ALU.mult,
                        )

            def transpose_pair(xns, dstT, evac_eng):
                """xns: two [P, TCH, C] bf16 tiles -> dstT [P, CCH, 2, T]."""
                for i in range(2):
                    trs = psp.tile([P, CCH, TCH, P], BF16, tag="ps",
                                   name=f"trs{i}")
                    for cc in range(CCH):
                        for tch in range(TCH):
                            nc.tensor.transpose(
                                trs[:, cc, tch, :],
                                xns[i][:, tch, cc * P:(cc + 1) * P],
                                ident[:],
                            )
                    eng = evac_eng[i % 2]
                    view = dstT[:, :, i, :].rearrange(
                        "p c (tc q) -> p c tc q", tc=TCH)
                    if eng == "act":
                        nc.scalar.copy(view, trs[:, :, :, :])
                    else:
                        nc.vector.tensor_copy(view, trs[:, :, :, :])

            state = {}

            def stage_A(pb):
                """x load, LN1, transpose, q/k/v projections for pair pb."""
                s = {}
                xts = []
                xns = []
                for i, b in enumerate((2 * pb, 2 * pb + 1)):
                    xt = wk_pool.tile([P, TCH, C], F32, tag=f"xt{i}", bufs=4)
                    nc.sync.dma_start(
                        xt[:], x_d[b].rearrange("(tc p) c -> p tc c", p=P))
                    xts.append(xt)
                for i in range(2):
                    xn = wk_pool.tile([P, TCH, C], BF16, tag=f"xn{i}",
                                      name=f"xn{i}")
                    xns.append(xn)
                layer_norm_pair(xts, xns, "ln1")
                xnT2 = wk_pool.tile([P, CCH, 2, T], FP8, tag="xnT2")
                transpose_pair(xns, xnT2, ("act", "dve"))

                qsb2 = wk_pool.tile([P, CCH, 2, T], BF16, tag="qsb2", bufs=3)
                ksb2 = wk_pool.tile([P, CCH, 2, T], BF16, tag="ksb2", bufs=3)
                xnT_dr = xnT2[:, 0:2, :, :].rearrange("p k i t -> p k (i t)")
                for wmat, bias_t, use_b, dst, eng in (
                    (wq, qb, use_qb, qsb2, "act"),
                    (wkk, kb, use_kb, ksb2, "dve"),
                ):
                    for mc in range(CCH):
                        pp = psp.tile([P, 2, T], F32, tag="ps", name="pp")
                        nc.tensor.matmul(
                            pp[:, :, :],
                            wmat[:, 0:2, mc * P:(mc + 1) * P],
                            xnT_dr,
                            start=True, stop=False, perf_mode=PM_DR)
                        nc.tensor.matmul(
                            pp[:, :, :],
                            wmat[:, 2, mc * P:(mc + 1) * P],
                            xnT2[:, 2, :, :],
                            start=False, stop=True)
                        if use_b:
                            nc.scalar.activation(
                                dst[:, mc, :, :], pp[:], AF.Identity,
                                bias=bias_t[:, mc:mc + 1], scale=1.0 / WSCL)
                        elif eng == "act":
                            nc.scalar.activation(
                                dst[:, mc, :, :], pp[:], AF.Identity,
                                scale=1.0 / WSCL)
                        else:
                            nc.vector.tensor_scalar_mul(
                                dst[:, mc, :, :], pp[:], 1.0 / WSCL)

                vsbs = []
                for i in range(2):
                    vsb = wk_pool.tile([P, TCH, HD], FP8, tag=f"vsb{i}", bufs=3)
                    vsbs.append(vsb)
                    for sc in range(TCH):
                        vp = psp.tile([P, HD], F32, tag="ps", name="vp")
                        nc.tensor.matmul(
                            vp[:, :],
                            xnT2[:, 0:2, i, sc * P:(sc + 1) * P],
                            wv[:, 0:2, :],
                            start=True, stop=False, perf_mode=PM_DR)
                        nc.tensor.matmul(
                            vp[:, :],
                            xnT2[:, 2, i, sc * P:(sc + 1) * P],
                            wv[:, 2, :],
                            start=False, stop=(not use_vb))
                        if use_vb:
                            nc.tensor.matmul(
                                vp[:, :], ones64[0:1, :], vb[0:1, :],
                                start=False, stop=True)
                        if sc == 0:
                            nc.scalar.activation(
                                vsb[:, sc, :], vp[:], AF.Identity,
                                scale=1.0 / WSCL)
                        else:
                            nc.vector.tensor_scalar_mul(
                                vsb[:, sc, :], vp[:], 1.0 / WSCL)
                s["xts"] = xts
                s["q"] = qsb2
                s["k"] = ksb2
                s["v"] = vsbs
                state[pb] = s

            def stage_B(pb):
                """attention, out-proj, residual, LN2+transpose for pair pb."""
                s = state[pb]
                qsb2, ksb2, vsbs, xts = s["q"], s["k"], s["v"], s["xts"]
                xnews = []
                xn2s = []
                osb2 = wk_pool.tile([P, HP, 2, T], BF16, tag="osb2")
                es = {}
                for i in range(2):
                    for hp in range(HP):
                        e = wk_pool.tile([P, 2, 3, P], FP8,
                                         tag=f"e{i}_{hp}")
                        es[i, hp] = e
                        for j in range(2):
                            off = 64 * j
                            kv = ksb2[off:off + D, hp, i, :]
                            qv = qsb2[off:off + D, hp, i, :]
                            sp = psp.tile([P, 3, P], F32, tag="ps",
                                          name=f"sp{j}")
                            # blocks [d0=(s0,t0), full=(s0,t1), d1=(s1,t1)]
                            nc.tensor.matmul(
                                sp[:, 0:2, :], kv[:, 0:P], qv[:, :],
                                start=True, stop=False)
                            nc.tensor.matmul(
                                sp[:, 2, :], kv[:, P:T], qv[:, P:T],
                                start=False, stop=False)
                            # adds -BIG*max(0, s-t) to the two diag blocks
                            nc.tensor.matmul(
                                sp[:, 0, :], trilm[:, :], negm2[:, 0, :],
                                start=False, stop=False)
                            nc.tensor.matmul(
                                sp[:, 2, :], trilm[:, :], negm2[:, 1, :],
                                start=False, stop=True)
                            nc.scalar.activation(
                                e[:, j], sp[:], AF.Exp, scale=SCALE)
                for hp in range(HP):
                    dp = psp.tile([P, 2, T], F32, tag="ps", name="dp")
                    op = psp.tile([P, 2, T], F32, tag="ps", name="op")
                    on64 = ones8[:, 0:64]
                    for j in range(2):
                        h = 2 * hp + j
                        po = 64 * j
                        for i in range(2):
                            e = es[i, hp]
                            st = (i == 0)
                            fin = (i == 1)
                            nc.tensor.matmul(
                                dp[po:po + 64, i, 0:P], on64,
                                e[:, j, 0, :], start=st, stop=False)
                            nc.tensor.matmul(
                                dp[po:po + 64, i, P:T], on64,
                                e[:, j, 1, :], start=False, stop=False)
                            nc.tensor.matmul(
                                dp[po:po + 64, i, P:T], on64,
                                e[:, j, 2, :], start=False, stop=fin)
                            vv0 = vsbs[i][:, 0, h * D:(h + 1) * D]
                            vv1 = vsbs[i][:, 1, h * D:(h + 1) * D]
                            nc.tensor.matmul(
                                op[po:po + 64, i, 0:P], vv0, e[:, j, 0, :],
                                start=st, stop=False)
                            nc.tensor.matmul(
                                op[po:po + 64, i, P:T], vv0, e[:, j, 1, :],
                                start=False, stop=False)
                            nc.tensor.matmul(
                                op[po:po + 64, i, P:T], vv1, e[:, j, 2, :],
                                start=False, stop=fin)
                    rbc = wk_pool.tile([P, 2, T], F32, tag=f"rbc{hp}")
                    nc.vector.reciprocal(rbc[:], dp[:])
                    nc.vector.tensor_tensor(
                        osb2[:, hp, :, :], op[:], rbc[:], ALU.mult)
                for i in range(2):
                    xnew = wk_pool.tile([P, TCH, C], F32, tag=f"xnew{i}")
                    xnews.append(xnew)
                    for tcc in range(TCH):
                        ap_t = psp.tile([P, C], F32, tag="ps", name="ap_t")
                        for hp in range(HP):
                            nc.tensor.matmul(
                                ap_t[:, :],
                                osb2[:, hp, i, tcc * P:(tcc + 1) * P],
                                wo[:, hp, :],
                                start=(hp == 0),
                                stop=(hp == HP - 1 and not use_bo))
                        if use_bo:
                            nc.tensor.matmul(
                                ap_t[:, :], ones64[0:1, :], bo[0:1, :],
                                start=False, stop=True)
                        nc.vector.tensor_tensor(
                            xnew[:, tcc, :], ap_t[:, :], xts[i][:, tcc, :],
                            ALU.add)
                    xn2 = wk_pool.tile([P, TCH, C], BF16, tag=f"xn2_{i}",
                                       name=f"xn2_{i}")
                    xn2s.append(xn2)
                layer_norm_pair(xnews, xn2s, "ln2")

                s["xnews"] = xnews
                s["xn2s"] = xn2s

            def stage_Bt(pb):
                """LN2 transposes for pair pb (emitted late so the LN2
                stats/apply chain hides under FFN matmuls)."""
                s = state[pb]
                xn2T2 = wk_pool.tile([P, CCH, 2, T], FP8, tag="xn2T2")
                transpose_pair(s.pop("xn2s"), xn2T2, ("dve", "act"))
                s["xn2T"] = xn2T2

            def stage_C(pb, mo_lo, mo_hi, store):
                """FFN chunk [mo_lo, mo_hi) + optional residual/store."""
                s = state[pb]
                xn2T2, xnews = s["xn2T"], s["xnews"]
                if mo_lo == 0:
                    s["fps"] = [psp.tile([P, C], F32, tag="ps", name=f"fp{j}")
                                for j in range(4)]
                fps = s["fps"]
                xn2T_dr = xn2T2[:, 0:2, :, :].rearrange("p k i t -> p k (i t)")
                hsm = None
                for mo in range(mo_lo, mo_hi):
                    hp2 = psp.tile([P, 2, T], F32, tag="ps", name="hp2")
                    nc.tensor.matmul(
                        hp2[:, :, :],
                        w1[:, 0:2, mo * P:(mo + 1) * P],
                        xn2T_dr,
                        start=True, stop=False, perf_mode=PM_DR)
                    nc.tensor.matmul(
                        hp2[:, :, :],
                        w1[:, 2, mo * P:(mo + 1) * P],
                        xn2T2[:, 2, :, :],
                        start=False, stop=True)
                    if mo % 2 == 0:
                        hsm = wk_pool.tile([P, 2, 2, T], FP8, tag="hsm",
                                           bufs=3)
                    if use_b1:
                        nc.scalar.activation(
                            hsm[:, mo % 2], hp2[:], AF.Relu,
                            bias=b1c[:, mo:mo + 1], scale=1.0 / WSCL)
                    else:
                        nc.scalar.activation(hsm[:, mo % 2], hp2[:], AF.Relu,
                                             scale=1.0 / WSCL)
                    if mo % 2 == 1:
                        mp = mo // 2
                        for i in range(2):
                            for tcc in range(TCH):
                                nc.tensor.matmul(
                                    fps[2 * i + tcc][:, :],
                                    hsm[:, :, i, tcc * P:(tcc + 1) * P],
                                    w2[:, mp, :, :],
                                    start=(mp == 0),
                                    stop=(mp == FCH // 2 - 1 and not use_b2),
                                    perf_mode=PM_DR)
                if not store:
                    return
                state.pop(pb)
                for i, b in enumerate((2 * pb, 2 * pb + 1)):
                    yout = wk_pool.tile([P, TCH, C], F32, tag=f"yout{i}")
                    for tcc in range(TCH):
                        fp = fps[2 * i + tcc]
                        if use_b2:
                            nc.tensor.matmul(
                                fp[:, :], ones64[0:1, :], b2[0:1, :],
                                start=False, stop=True)
                        nc.vector.scalar_tensor_tensor(
                            yout[:, tcc, :], fp[:, :], 1.0 / WSCL,
                            xnews[i][:, tcc, :], ALU.mult, ALU.add)
                    nc.sync.dma_start(
                        y_d[b].rearrange("(tc p) c -> p tc c", p=P),
                        yout[:])

            def body():
                # pipelined emission; C is split in halves so the LN2/LN1
                # DVE chain of B(pb) hides under FFN matmuls of C(pb-1).
                MH = FCH // 2
                stage_A(0)
                if npairs > 1:
                    stage_A(1)
                if npairs > 2:
                    stage_A(2)
                stage_B(0)
                stage_Bt(0)
                for pb in range(1, npairs):
                    if pb + 2 < npairs:
                        stage_A(pb + 2)
                    stage_C(pb - 1, 0, MH, store=False)
                    stage_B(pb)
                    stage_C(pb - 1, MH, FCH, store=True)
                    stage_Bt(pb)
                stage_C(npairs - 1, 0, FCH, store=True)

            if repeat > 1:
                with tc.For_i(0, repeat, 1):
                    body()
            else:
                body()

    nc.compile()
    return nc


def _make_negm2():
    f32 = np.float32
    jgt = np.tril(np.ones((P, P), dtype=f32), -1)  # [j, t] = 1 iff j > t
    m = np.zeros((P, 2, P), dtype=f32)
    m[:, 0, :] = -BIG * jgt
    m[:, 1, :] = -BIG * jgt
    return m


def prep_weights2(Wq, Wk, Wv, Wo, bo, W1, b1, W2, b2, g1, be1, g2, be2):
    """Fold LN gamma/beta into weights; rearrange + quantize to bf16."""
    import ml_dtypes
    bf16 = ml_dtypes.bfloat16
    f32 = np.float32

    def kchunk(w, kdim):  # [K, M] -> [P, K//P, M]
        m = w.shape[1]
        return np.ascontiguousarray(
            np.asarray(w, f32).reshape(kdim // P, P, m).transpose(1, 0, 2))

    Wq2 = Wq.transpose(1, 0, 2).reshape(C, HD)
    Wk2 = Wk.transpose(1, 0, 2).reshape(C, HD)
    Wv2 = Wv.transpose(1, 0, 2).reshape(C, HD)
    import ml_dtypes as _mld
    fp8 = _mld.float8_e4m3
    WS = 32.0
    out = {
        "wq": (WS * kchunk(g1[:, None] * Wq2, C)).astype(fp8),
        "wk": (WS * kchunk(g1[:, None] * Wk2, C)).astype(fp8),
        "wv": (WS * kchunk(g1[:, None] * Wv2, C)).astype(fp8),
        # wo[p, hp, c] = Wo[hp*128 + p, c]
        "wo": kchunk(Wo, HD).astype(bf16),
        "w1": (WS * kchunk(g2[:, None] * W1, C)).astype(fp8),
        # w2_dr[p, mp, i, c] = WS * W2[(2mp+i)*128 + p, c]
        "w2": (WS * kchunk(W2, F).reshape(P, FCH // 2, 2, C)).astype(fp8),
        "ident": np.eye(P, dtype=f32).astype(bf16),
        "trilm": np.tril(np.ones((P, P), dtype=f32)).T.copy().astype(bf16),
        "negm2": _make_negm2().astype(bf16),
        "ones64": np.ones((P, P), dtype=f32).astype(bf16),
        "ones8": np.ones((P, P), dtype=f32).astype(fp8),
    }
    qbv = be1 @ Wq2
    kbv = be1 @ Wk2
    vbv = be1 @ Wv2
    b1e = be2 @ W1 + b1
    out["qb"] = np.ascontiguousarray(qbv.reshape(CCH, P).T).astype(f32)
    out["kb"] = np.ascontiguousarray(kbv.reshape(CCH, P).T).astype(f32)
    out["vb"] = (WS * vbv)[None, :].astype(bf16)
    out["bo"] = np.asarray(bo)[None, :].astype(bf16)
    out["b1c"] = np.ascontiguousarray(b1e.reshape(FCH, P).T).astype(f32)
    out["b2"] = (WS * np.asarray(b2))[None, :].astype(bf16)
    flags = set()
    for name, vec in (("qb", qbv), ("kb", kbv), ("vb", vbv),
                      ("bo", np.asarray(bo)), ("b1", b1e),
                      ("b2", np.asarray(b2))):
        if np.any(np.asarray(vec) != 0):
            flags.add(name)
    return out, frozenset(flags)


_PROGRAM_CACHE = {}


def _get_program(bl, flags):
    key = (bl, flags)
    if key not in _PROGRAM_CACHE:
        _PROGRAM_CACHE[key] = build_program2(bl, flags)
    return _PROGRAM_CACHE[key]


def kernel(x, Wq, Wk, Wv, Wo, bo, W1, b1, W2, b2, g1, be1, g2, be2, **kw):
    from concourse.bass_utils import run_bass_kernel_spmd

    args = [np.asarray(a, dtype=np.float32) for a in
            (x, Wq, Wk, Wv, Wo, bo, W1, b1, W2, b2, g1, be1, g2, be2)]
    x = args[0]
    wmap, flags = prep_weights2(*args[1:])
    nc = _get_program(BL, flags)
    xs = x.reshape(NCORES, BL, T, C)
    in_maps = []
    for c in range(NCORES):
        m = {"x": np.ascontiguousarray(xs[c])}
        m.update(wmap)
        in_maps.append(m)
    res = run_bass_kernel_spmd(nc, in_maps, list(range(NCORES)), **kw)
    y = np.stack([res.results[i]["y"] for i in range(NCORES)], axis=0)
    return y.reshape(B, T, C)


# revision 3
# speedup vs baseline: 1.0459x; 1.0282x over previous
"""Trainium2 Bass kernel v2 for the pre-LN transformer block
(B=128,T=256,C=384,H=6,D=64), data-parallel over batch across 8 cores.

Differences vs v1:
- All matmul operands are bf16 (PSUM stays f32, residual stream f32).
  bf16 runs at 1 cycle/row at any moving width (fp32r needs >=256) and
  halves SBUF for weights and activations.
- Causal block sparsity: with T=256 split in two 128-chunks, the
  (s-chunk 1, t-chunk 0) score block is fully masked and never computed.
  Per head the scores are 3 blocks [d0=(s0,t0) diag, full=(s0,t1),
  d1=(s1,t1) diag]; the diag blocks get the -BIG upper-triangle mask from
  one matmul whose middle moving block is zero.
- Heads processed in pairs: head 2j occupies PSUM partitions 0:64 and
  2j+1 partitions 64:128 of shared denominator / attn@v accumulators, so
  the output projection contracts K=128 (3 matmuls instead of 6).
- Software pipelining: emission order A0 A1 B0 A2 C0 B1 A3 C1 ... where
  A=load+LN1+QKV, B=attention+LN2, C=FFN+store. Engines execute in-order,
  so this fills LN pipeline bubbles with independent work.
"""

import sys

if "/opt/trn_rl_repo" not in sys.path:
    sys.path.insert(0, "/opt/trn_rl_repo")

import numpy as np

import concourse.bass as bass
import concourse.mybir as mybir
import concourse.tile as tile
from concourse import bacc

_KEEP_ACT_SET = "natural_log_exp_and_others"
_orig_get_act_tables = bacc.get_activation_tables


def _one_set_tables(arch):
    t = _orig_get_act_tables(arch)
    assert _KEEP_ACT_SET in t
    return {k: (v if k == _KEEP_ACT_SET else set()) for k, v in t.items()}


bacc.get_activation_tables = _one_set_tables

F32 = mybir.dt.float32
BF16 = mybir.dt.bfloat16
FP8 = mybir.dt.float8e4
WSCL = 32.0               # fp8 weights are pre-scaled by this; descaled at evac
PM_DR = mybir.MatmulPerfMode.DoubleRow
AF = mybir.ActivationFunctionType
ALU = mybir.AluOpType

B, T, C, H, D = 128, 256, 384, 6, 64
NCORES = 8
BL = B // NCORES
F = 4 * C
P = 128
TCH = T // P              # 2
CCH = C // P              # 3
FCH = F // P              # 12
HD = H * D
HP = H // 2               # head pairs
SCALE = float(C) ** -0.5
EPS = 1e-5
BIG = 1e30


def build_program2(bl=BL, flags=frozenset(), repeat=1,
                   interp_safe=False):
    assert bl % 2 == 0
    npairs = bl // 2
    use_qb = "qb" in flags
    use_kb = "kb" in flags
    use_vb = "vb" in flags
    use_bo = "bo" in flags
    use_b1 = "b1" in flags
    use_b2 = "b2" in flags

    nc = bacc.Bacc("TRN2", target_bir_lowering=False, debug=False,
                   num_devices=NCORES)

    x_d = nc.dram_tensor("x", [bl, T, C], F32, kind="ExternalInput")
    wq_d = nc.dram_tensor("wq", [P, CCH, HD], FP8, kind="ExternalInput")
    wk_d = nc.dram_tensor("wk", [P, CCH, HD], FP8, kind="ExternalInput")
    wv_d = nc.dram_tensor("wv", [P, CCH, HD], FP8, kind="ExternalInput")
    wo_d = nc.dram_tensor("wo", [P, HP, C], BF16, kind="ExternalInput")
    w1_d = nc.dram_tensor("w1", [P, CCH, F], FP8, kind="ExternalInput")
    w2_d = nc.dram_tensor("w2", [P, FCH // 2, 2, C], FP8, kind="ExternalInput")
    id_d = nc.dram_tensor("ident", [P, P], BF16, kind="ExternalInput")
    tl_d = nc.dram_tensor("trilm", [P, P], BF16, kind="ExternalInput")
    ng_d = nc.dram_tensor("negm2", [P, 2, P], BF16, kind="ExternalInput")
    on_d = nc.dram_tensor("ones64", [P, P], BF16, kind="ExternalInput")
    on8_d = nc.dram_tensor("ones8", [P, P], FP8, kind="ExternalInput")
    qb_d = nc.dram_tensor("qb", [P, CCH], F32, kind="ExternalInput")
    kb_d = nc.dram_tensor("kb", [P, CCH], F32, kind="ExternalInput")
    vb_d = nc.dram_tensor("vb", [1, HD], BF16, kind="ExternalInput")
    bo_d = nc.dram_tensor("bo", [1, C], BF16, kind="ExternalInput")
    b1_d = nc.dram_tensor("b1c", [P, FCH], F32, kind="ExternalInput")
    b2_d = nc.dram_tensor("b2", [1, C], BF16, kind="ExternalInput")
    y_d = nc.dram_tensor("y", [bl, T, C], F32, kind="ExternalOutput")

    with tile.TileContext(nc) as tc:
        with (
            tc.tile_pool(name="wpool", bufs=1) as wp,
            tc.tile_pool(name="work", bufs=2) as wk_pool,
            tc.tile_pool(name="ps", bufs=8, space="PSUM") as psp,
        ):
            # ---- constants / weights (loaded once, in first-use order) ----
            ident = wp.tile([P, P], BF16)
            wq = wp.tile([P, CCH, HD], FP8)
            wkk = wp.tile([P, CCH, HD], FP8)
            wv = wp.tile([P, CCH, HD], FP8)
            trilm = wp.tile([P, P], BF16)
            negm2 = wp.tile([P, 2, P], BF16)
            ones64 = wp.tile([P, P], BF16)
            ones8 = wp.tile([P, P], FP8)
            wo = wp.tile([P, HP, C], BF16)
            w1 = wp.tile([P, CCH, F], FP8)
            w2 = wp.tile([P, FCH // 2, 2, C], FP8)
            epsb = wp.tile([P, 1], F32)
            nc.gpsimd.memset(epsb[:], EPS)
            for dst, src in ((ident, id_d), (wq, wq_d), (wkk, wk_d),
                             (wv, wv_d), (trilm, tl_d), (negm2, ng_d),
                             (ones64, on_d), (ones8, on8_d),
                             (wo, wo_d), (w1, w1_d),
                             (w2, w2_d)):
                nc.gpsimd.dma_start(dst[:], src[:])
            qb = kb = vb = bo = b1c = b2 = None
            if use_qb:
                qb = wp.tile([P, CCH], F32)
                nc.sync.dma_start(qb[:], qb_d[:])
            if use_kb:
                kb = wp.tile([P, CCH], F32)
                nc.sync.dma_start(kb[:], kb_d[:])
            if use_vb:
                vb = wp.tile([1, HD], BF16)
                nc.sync.dma_start(vb[:], vb_d[:])
            if use_bo:
                bo = wp.tile([1, C], BF16)
                nc.sync.dma_start(bo[:], bo_d[:])
            if use_b1:
                b1c = wp.tile([P, FCH], F32)
                nc.sync.dma_start(b1c[:], b1_d[:])
            if use_b2:
                b2 = wp.tile([1, C], BF16)
                nc.sync.dma_start(b2[:], b2_d[:])

            def layer_norm_pair(srcs, xns, tag):
                """token-major LN for both batches of a pair:
                xn (bf16) = (src - mu) * rstd."""
                st6 = wk_pool.tile([P, 2, TCH, 6], F32, tag=f"st6_{tag}")
                mv = wk_pool.tile([P, 2, TCH, 2], F32, tag=f"mv_{tag}")
                rstd = wk_pool.tile([P, 2, TCH], F32, tag=f"rstd_{tag}")
                for i in range(2):
                    for tch in range(TCH):
                        nc.vector.bn_stats(st6[:, i, tch, :],
                                           srcs[i][:, tch, :])
                        nc.vector.bn_aggr(mv[:, i, tch, :],
                                          st6[:, i, tch, :])
                nc.scalar.activation(rstd[:], mv[:, :, :, 1], AF.Ln,
                                     bias=epsb[:])
                nc.scalar.activation(rstd[:], rstd[:], AF.Exp, scale=-0.5)
                for i in range(2):
                    for tch in range(TCH):
                        nc.vector.tensor_scalar(
                            xns[i][:, tch, :], srcs[i][:, tch, :],
                            mv[:, i, tch, 0:1], rstd[:, i, tch:tch + 1],
                            ALU.subtract, ALU.mult,
                        )

            def transpose_pair(xns, dstT, evac_eng):
                """xns: two [P, TCH, C] bf16 tiles -> dstT [P, CCH, 2, T]."""
                for i in range(2):
                    trs = psp.tile([P, CCH, TCH, P], BF16, tag="ps",
                                   name=f"trs{i}")
                    for cc in range(CCH):
                        for tch in range(TCH):
                            nc.tensor.transpose(
                                trs[:, cc, tch, :],
                                xns[i][:, tch, cc * P:(cc + 1) * P],
                                ident[:],
                            )
                    eng = evac_eng[i % 2]
                    view = dstT[:, :, i, :].rearrange(
                        "p c (tc q) -> p c tc q", tc=TCH)
                    if eng == "act":
                        nc.scalar.copy(view, trs[:, :, :, :])
                    else:
                        nc.vector.tensor_copy(view, trs[:, :, :, :])

            state = {}

            def stage_A(pb):
                """x load, LN1, transpose, q/k/v projections for pair pb."""
                s = {}
                xts = []
                xns = []
                for i, b in enumerate((2 * pb, 2 * pb + 1)):
                    xt = wk_pool.tile([P, TCH, C], F32, tag=f"xt{i}", bufs=4)
                    nc.sync.dma_start(
                        xt[:], x_d[b].rearrange("(tc p) c -> p tc c", p=P))
                    xts.append(xt)
                for i in range(2):
                    xn = wk_pool.tile([P, TCH, C], BF16, tag=f"xn{i}",
                                      name=f"xn{i}")
                    xns.append(xn)
                layer_norm_pair(xts, xns, "ln1")
                xnT2 = wk_pool.tile([P, CCH, 2, T], FP8, tag="xnT2")
                transpose_pair(xns, xnT2, ("act", "dve"))

                qsb2 = wk_pool.tile([P, CCH, 2, T], BF16, tag="qsb2", bufs=3)
                ksb2 = wk_pool.tile([P, CCH, 2, T], BF16, tag="ksb2", bufs=3)
                xnT_dr = xnT2[:, 0:2, :, :].rearrange("p k i t -> p k (i t)")
                for wmat, bias_t, use_b, dst, eng in (
                    (wq, qb, use_qb, qsb2, "act"),
                    (wkk, kb, use_kb, ksb2, "dve"),
                ):
                    for mc in range(CCH):
                        pp = psp.tile([P, 2, T], F32, tag="ps", name="pp")
                        nc.tensor.matmul(
                            pp[:, :, :],
                            wmat[:, 0:2, mc * P:(mc + 1) * P],
                            xnT_dr,
                            start=True, stop=False, perf_mode=PM_DR)
                        nc.tensor.matmul(
                            pp[:, :, :],
                            wmat[:, 2, mc * P:(mc + 1) * P],
                            xnT2[:, 2, :, :],
                            start=False, stop=True)
                        if use_b:
                            nc.scalar.activation(
                                dst[:, mc, :, :], pp[:], AF.Identity,
                                bias=bias_t[:, mc:mc + 1], scale=1.0 / WSCL)
                        elif eng == "act":
                            nc.scalar.activation(
                                dst[:, mc, :, :], pp[:], AF.Identity,
                                scale=1.0 / WSCL)
                        else:
                            nc.vector.tensor_scalar_mul(
                                dst[:, mc, :, :], pp[:], 1.0 / WSCL)

                vsbs = []
                for i in range(2):
                    vsb = wk_pool.tile([P, TCH, HD], FP8, tag=f"vsb{i}", bufs=3)
                    vsbs.append(vsb)
                    for sc in range(TCH):
                        vp = psp.tile([P, HD], F32, tag="ps", name="vp")
                        nc.tensor.matmul(
                            vp[:, :],
                            xnT2[:, 0:2, i, sc * P:(sc + 1) * P],
                            wv[:, 0:2, :],
                            start=True, stop=False, perf_mode=PM_DR)
                        nc.tensor.matmul(
                            vp[:, :],
                            xnT2[:, 2, i, sc * P:(sc + 1) * P],
                            wv[:, 2, :],
                            start=False, stop=(not use_vb))
                        if use_vb:
                            nc.tensor.matmul(
                                vp[:, :], ones64[0:1, :], vb[0:1, :],
                                start=False, stop=True)
                        if sc == 0:
                            nc.scalar.activation(
                                vsb[:, sc, :], vp[:], AF.Identity,
                                scale=1.0 / WSCL)
                        else:
                            nc.vector.tensor_scalar_mul(
                                vsb[:, sc, :], vp[:], 1.0 / WSCL)
                s["xts"] = xts
                s["q"] = qsb2
                s["k"] = ksb2
                s["v"] = vsbs
                state[pb] = s

            def stage_B_scores(pb, hp):
                """scores + exp for head-pair hp, both batches."""
                s = state[pb]
                qsb2, ksb2 = s["q"], s["k"]
                es = s.setdefault("es", {})
                if hp == 0:
                    s["osb2"] = wk_pool.tile([P, HP, 2, T], BF16,
                                             tag="osb2", name="osb2")
                for i in range(2):
                    if True:
                        e = wk_pool.tile([P, 2, 3, P], FP8,
                                         tag=f"e{i}_{hp}")
                        es[i, hp] = e
                        for j in range(2):
                            off = 64 * j
                            kv = ksb2[off:off + D, hp, i, :]
                            qv = qsb2[off:off + D, hp, i, :]
                            sp = psp.tile([P, 3, P], F32, tag="ps",
                                          name=f"sp{j}")
                            # blocks [d0=(s0,t0), full=(s0,t1), d1=(s1,t1)]
                            nc.tensor.matmul(
                                sp[:, 0:2, :], kv[:, 0:P], qv[:, :],
                                start=True, stop=False)
                            nc.tensor.matmul(
                                sp[:, 2, :], kv[:, P:T], qv[:, P:T],
                                start=False, stop=False)
                            # adds -BIG*max(0, s-t) to the two diag blocks
                            if interp_safe:
                                nc.tensor.matmul(
                                    sp[:, 0, :], trilm[:, :], negm2[:, 0, :],
                                    start=False, stop=False)
                                nc.tensor.matmul(
                                    sp[:, 2, :], trilm[:, :], negm2[:, 1, :],
                                    start=False, stop=True)
                            else:
                                nc.tensor.matmul(
                                    sp[:, 0::2, :], trilm[:, :],
                                    negm2[:, :, :],
                                    start=False, stop=True)
                            nc.scalar.activation(
                                e[:, j], sp[:], AF.Exp, scale=SCALE)
            def stage_B_dpop(pb, hp):
                """denominator + attn@v + normalize for head-pair hp."""
                s = state[pb]
                vsbs, es, osb2 = s["v"], s["es"], s["osb2"]
                if True:
                    dp = psp.tile([P, 2, T], F32, tag="ps", name="dp")
                    op = psp.tile([P, 2, T], F32, tag="ps", name="op")
                    on64 = ones8[:, 0:64]
                    for j in range(2):
                        h = 2 * hp + j
                        po = 64 * j
                        for i in range(2):
                            e = es[i, hp]
                            st = (i == 0)
                            fin = (i == 1)
                            nc.tensor.matmul(
                                dp[po:po + 64, i, 0:T], on64,
                                e[:, j, 0:2, :], start=st, stop=False)
                            nc.tensor.matmul(
                                dp[po:po + 64, i, P:T], on64,
                                e[:, j, 2, :], start=False, stop=fin)
                            vv0 = vsbs[i][:, 0, h * D:(h + 1) * D]
                            vv1 = vsbs[i][:, 1, h * D:(h + 1) * D]
                            nc.tensor.matmul(
                                op[po:po + 64, i, 0:T], vv0,
                                e[:, j, 0:2, :], start=st, stop=False)
                            nc.tensor.matmul(
                                op[po:po + 64, i, P:T], vv1, e[:, j, 2, :],
                                start=False, stop=fin)
                    rbc = wk_pool.tile([P, 2, T], F32, tag=f"rbc{hp}")
                    nc.vector.reciprocal(rbc[:], dp[:])
                    nc.vector.tensor_tensor(
                        osb2[:, hp, :, :], op[:], rbc[:], ALU.mult)
            def stage_B_tail(pb):
                """out-projection, residual, LN2 stats/apply."""
                s = state[pb]
                xts, osb2 = s["xts"], s["osb2"]
                xnews = []
                xn2s = []
                for i in range(2):
                    xnew = wk_pool.tile([P, TCH, C], F32, tag=f"xnew{i}")
                    xnews.append(xnew)
                    for tcc in range(TCH):
                        ap_t = psp.tile([P, C], F32, tag="ps", name="ap_t")
                        for hp in range(HP):
                            nc.tensor.matmul(
                                ap_t[:, :],
                                osb2[:, hp, i, tcc * P:(tcc + 1) * P],
                                wo[:, hp, :],
                                start=(hp == 0),
                                stop=(hp == HP - 1 and not use_bo))
                        if use_bo:
                            nc.tensor.matmul(
                                ap_t[:, :], ones64[0:1, :], bo[0:1, :],
                                start=False, stop=True)
                        nc.vector.tensor_tensor(
                            xnew[:, tcc, :], ap_t[:, :], xts[i][:, tcc, :],
                            ALU.add)
                    xn2 = wk_pool.tile([P, TCH, C], BF16, tag=f"xn2_{i}",
                                       name=f"xn2_{i}")
                    xn2s.append(xn2)
                layer_norm_pair(xnews, xn2s, "ln2")

                s["xnews"] = xnews
                s["xn2s"] = xn2s

            def stage_Bt(pb):
                """LN2 transposes for pair pb (emitted late so the LN2
                stats/apply chain hides under FFN matmuls)."""
                s = state[pb]
                xn2T2 = wk_pool.tile([P, CCH, 2, T], FP8, tag="xn2T2")
                transpose_pair(s.pop("xn2s"), xn2T2, ("dve", "act"))
                s["xn2T"] = xn2T2

            def stage_C(pb, mo_lo, mo_hi, store):
                """FFN chunk [mo_lo, mo_hi) + optional residual/store."""
                s = state[pb]
                xn2T2, xnews = s["xn2T"], s["xnews"]
                if mo_lo == 0:
                    s["fps"] = [psp.tile([P, C], F32, tag="ps", name=f"fp{j}")
                                for j in range(4)]
                fps = s["fps"]
                xn2T_dr = xn2T2[:, 0:2, :, :].rearrange("p k i t -> p k (i t)")
                hsm = None
                for mo in range(mo_lo, mo_hi):
                    hp2 = psp.tile([P, 2, T], F32, tag="ps", name="hp2")
                    nc.tensor.matmul(
                        hp2[:, :, :],
                        w1[:, 0:2, mo * P:(mo + 1) * P],
                        xn2T_dr,
                        start=True, stop=False, perf_mode=PM_DR)
                    nc.tensor.matmul(
                        hp2[:, :, :],
                        w1[:, 2, mo * P:(mo + 1) * P],
                        xn2T2[:, 2, :, :],
                        start=False, stop=True)
                    if mo % 2 == 0:
                        hsm = wk_pool.tile([P, 2, 2, T], FP8, tag="hsm",
                                           bufs=3)
                    if mo % 3 == 1:
                        if use_b1:
                            nc.vector.tensor_scalar(
                                hsm[:, mo % 2], hp2[:],
                                b1c[:, mo:mo + 1], 0.0, ALU.add, ALU.max)
                        else:
                            nc.vector.tensor_scalar_max(
                                hsm[:, mo % 2], hp2[:], 0.0)
                    elif use_b1:
                        nc.scalar.activation(
                            hsm[:, mo % 2], hp2[:], AF.Relu,
                            bias=b1c[:, mo:mo + 1])
                    else:
                        nc.scalar.activation(hsm[:, mo % 2], hp2[:], AF.Relu)
                    if mo % 2 == 1:
                        mp = mo // 2
                        for i in range(2):
                            for tcc in range(TCH):
                                nc.tensor.matmul(
                                    fps[2 * i + tcc][:, :],
                                    hsm[:, :, i, tcc * P:(tcc + 1) * P],
                                    w2[:, mp, :, :],
                                    start=(mp == 0),
                                    stop=(mp == FCH // 2 - 1 and not use_b2),
                                    perf_mode=PM_DR)
                if not store:
                    return
                state.pop(pb)
                for i, b in enumerate((2 * pb, 2 * pb + 1)):
                    yout = wk_pool.tile([P, TCH, C], F32, tag=f"yout{i}")
                    for tcc in range(TCH):
                        fp = fps[2 * i + tcc]
                        if use_b2:
                            nc.tensor.matmul(
                                fp[:, :], ones64[0:1, :], b2[0:1, :],
                                start=False, stop=True)
                        nc.vector.scalar_tensor_tensor(
                            yout[:, tcc, :], fp[:, :], 1.0 / (WSCL * WSCL),
                            xnews[i][:, tcc, :], ALU.mult, ALU.add)
                    nc.sync.dma_start(
                        y_d[b].rearrange("(tc p) c -> p tc c", p=P),
                        yout[:])

            def body():
                # pipelined emission; C is split in halves so the LN2/LN1
                # DVE chain of B(pb) hides under FFN matmuls of C(pb-1).
                def stage_B_all(pb):
                    for hp in range(HP):
                        stage_B_scores(pb, hp)
                        stage_B_dpop(pb, hp)
                    stage_B_tail(pb)

                stage_A(0)
                if npairs > 1:
                    stage_A(1)
                if npairs > 2:
                    stage_A(2)
                stage_B_all(0)
                stage_Bt(0)
                for pb in range(1, npairs):
                    if pb + 2 < npairs:
                        stage_A(pb + 2)
                    stage_C(pb - 1, 0, 2, store=False)
                    stage_B_scores(pb, 0)
                    stage_C(pb - 1, 2, 4, store=False)
                    stage_B_dpop(pb, 0)
                    stage_B_scores(pb, 1)
                    stage_C(pb - 1, 4, 6, store=False)
                    stage_B_dpop(pb, 1)
                    stage_B_scores(pb, 2)
                    stage_C(pb - 1, 6, 8, store=False)
                    stage_B_dpop(pb, 2)
                    stage_C(pb - 1, 8, 10, store=False)
                    stage_B_tail(pb)
                    stage_C(pb - 1, 10, 12, store=True)
                    stage_Bt(pb)
                stage_C(npairs - 1, 0, FCH, store=True)

            if repeat > 1:
                with tc.For_i(0, repeat, 1):
                    body()
            else:
                body()

    nc.compile()
    return nc


def _make_negm2():
    f32 = np.float32
    jgt = np.tril(np.ones((P, P), dtype=f32), -1)  # [j, t] = 1 iff j > t
    m = np.zeros((P, 2, P), dtype=f32)
    m[:, 0, :] = -BIG * jgt
    m[:, 1, :] = -BIG * jgt
    return m


def prep_weights2(Wq, Wk, Wv, Wo, bo, W1, b1, W2, b2, g1, be1, g2, be2):
    """Fold LN gamma/beta into weights; rearrange + quantize to bf16."""
    import ml_dtypes
    bf16 = ml_dtypes.bfloat16
    f32 = np.float32

    def kchunk(w, kdim):  # [K, M] -> [P, K//P, M]
        m = w.shape[1]
        return np.ascontiguousarray(
            np.asarray(w, f32).reshape(kdim // P, P, m).transpose(1, 0, 2))

    Wq2 = Wq.transpose(1, 0, 2).reshape(C, HD)
    Wk2 = Wk.transpose(1, 0, 2).reshape(C, HD)
    Wv2 = Wv.transpose(1, 0, 2).reshape(C, HD)
    import ml_dtypes as _mld
    fp8 = _mld.float8_e4m3
    WS = 32.0
    out = {
        "wq": (WS * kchunk(g1[:, None] * Wq2, C)).astype(fp8),
        "wk": (WS * kchunk(g1[:, None] * Wk2, C)).astype(fp8),
        "wv": (WS * kchunk(g1[:, None] * Wv2, C)).astype(fp8),
        # wo[p, hp, c] = Wo[hp*128 + p, c]
        "wo": kchunk(Wo, HD).astype(bf16),
        "w1": (WS * kchunk(g2[:, None] * W1, C)).astype(fp8),
        # w2_dr[p, mp, i, c] = WS * W2[(2mp+i)*128 + p, c]
        "w2": (WS * kchunk(W2, F).reshape(P, FCH // 2, 2, C)).astype(fp8),
        "ident": np.eye(P, dtype=f32).astype(bf16),
        "trilm": np.tril(np.ones((P, P), dtype=f32)).T.copy().astype(bf16),
        "negm2": _make_negm2().astype(bf16),
        "ones64": np.ones((P, P), dtype=f32).astype(bf16),
        "ones8": np.ones((P, P), dtype=f32).astype(fp8),
    }
    qbv = be1 @ Wq2
    kbv = be1 @ Wk2
    vbv = be1 @ Wv2
    b1e = be2 @ W1 + b1
    out["qb"] = np.ascontiguousarray(qbv.reshape(CCH, P).T).astype(f32)
    out["kb"] = np.ascontiguousarray(kbv.reshape(CCH, P).T).astype(f32)
    out["vb"] = (WS * vbv)[None, :].astype(bf16)
    out["bo"] = np.asarray(bo)[None, :].astype(bf16)
    out["b1c"] = np.ascontiguousarray(
        WS * b1e.reshape(FCH, P).T).astype(f32)
    out["b2"] = (WS * np.asarray(b2))[None, :].astype(bf16)
    flags = set()
    for name, vec in (("qb", qbv), ("kb", kbv), ("vb", vbv),
                      ("bo", np.asarray(bo)), ("b1", b1e),
                      ("b2", np.asarray(b2))):
        if np.any(np.asarray(vec) != 0):
            flags.add(name)
    return out, frozenset(flags)


_PROGRAM_CACHE = {}


def _get_program(bl, flags):
    key = (bl, flags)
    if key not in _PROGRAM_CACHE:
        _PROGRAM_CACHE[key] = build_program2(bl, flags)
    return _PROGRAM_CACHE[key]


def kernel(x, Wq, Wk, Wv, Wo, bo, W1, b1, W2, b2, g1, be1, g2, be2, **kw):
    from concourse.bass_utils import run_bass_kernel_spmd

    args = [np.asarray(a, dtype=np.float32) for a in
            (x, Wq, Wk, Wv, Wo, bo, W1, b1, W2, b2, g1, be1, g2, be2)]
    x = args[0]
    wmap, flags = prep_weights2(*args[1:])
    nc = _get_program(BL, flags)
    xs = x.reshape(NCORES, BL, T, C)
    in_maps = []
    for c in range(NCORES):
        m = {"x": np.ascontiguousarray(xs[c])}
        m.update(wmap)
        in_maps.append(m)
    res = run_bass_kernel_spmd(nc, in_maps, list(range(NCORES)), **kw)
    y = np.stack([res.results[i]["y"] for i in range(NCORES)], axis=0)
    return y.reshape(B, T, C)
